# revision 38
# baseline (speedup 1.0000x reference)
"""Trainium2 Bass kernel for a binarized (1w1a) BasicBlock:

    out = BN2(PReLU(conv3x3(sign(x1), std2*sign(W2)) + b2)) + x1
    x1  = BN1(PReLU(conv3x3(sign(x),  std1*sign(W1)) + b1)) + x

Strategy
--------
Data-parallel over the batch axis: each of the 8 NeuronCores processes 8 of
the 64 images, with the (small) weights / BN / PReLU params replicated.
No collectives are needed.

Per-core compute:
  * Activations and weights are binarized (sign -> {-1, 0, +1} or the
    (w>0)-0.5 = +-0.5 trick), exactly representable in fp8e4m3.  The 3x3
    conv over 256->256 channels is 9 shifted matmuls accumulating in PSUM,
    using the fp8 DoubleRow perf mode so each matmul contracts the full
    K=256 input channels (2 fp8 weights per PE cell).
  * Pad-row-free strip layout: per partition, super-row r (r = 0..31)
    concatenates row r of all 8 images, each 33 wide (32 real + 1 zero pad
    col); 32 super-rows of 264 form one 8448-element strip per channel
    half, with zero guards on both ends.  Vertical 3x3 taps are +-264,
    horizontal +-1, so every tap is a constant offset and a 396-element
    window (<= 1 PSUM bank) is one uniform-stride matmul.  No pad rows:
    3.1% layout overhead instead of 6.3%.
  * PSUM evacuation folds conv-bias + BN scale into one ScalarE activation.
    PReLU is one fused VectorE op: max(alpha*t, t).  The post-BN shifts of
    BOTH convs are folded into conv1's residual add (o1 stores out1 + d2),
    so conv2's residual is a plain add and sign(out1) = sign(o1 - d2), one
    ScalarE activation.  conv1's chain stays f32 (any rounding feeding the
    sign flips decisions near threshold, ~1e-2 rel err); conv2's chain and
    the output run in bf16 (output-only path, 2x DVE rate, half DMA).
  * o1 is allocated per-pair so post-op chains of different pairs never
    serialize through one tile's dependency tracking.  Conv groups are
    single pairs (4 PSUM banks), giving 2-deep PSUM pipelining.  The conv2
    input strip is split in 3 tiles so its late windows only wait on the
    conv1 pairs that feed them.  GpSimd (slow DSP) only issues x DMAs;
    weight DMAs go through the SP queue in parallel, the first conv1
    weights in 3-tap chunks so the first matmul starts as early as
    possible.

The host side only reshapes/transposes/zero-pads/casts (layout), shards the
batch and un-packs the output strip.  All arithmetic (sign, BN folding,
conv, PReLU, residual) happens on-device.
"""

import math
import os
import sys

import numpy as np

for _p in ("/opt/trn_rl_repo", "/root/.axon_site/_ro/trn_rl_repo"):
    if os.path.isdir(_p) and _p not in sys.path:
        sys.path.insert(0, _p)

import concourse.bass as bass
import concourse.bacc as bacc
import concourse.mybir as mybir
from concourse import tile
from concourse.bass_utils import run_bass_kernel_spmd

F32 = mybir.dt.float32
BF16 = mybir.dt.bfloat16
F8 = mybir.dt.float8e4
AOP = mybir.AluOpType
AFT = mybir.ActivationFunctionType
DR = mybir.MatmulPerfMode.DoubleRow

EPS = 1e-5
NCORES = 8
NIMG = 8            # images per core
NCOL = 33           # cols per image row (32 real + 1 pad)
SR = NIMG * NCOL    # super-row length: row r of all 8 images  (264)
NSR = 32            # super-rows per strip
STRIP = NSR * SR    # 8448
GF = 272            # front guard (>= SR + 1, 16-aligned)
WCOLS = 396         # window: 1.5 super-rows (<= 512 PSUM bank)
STD = math.sqrt(2.0) / math.sqrt(256 * 9)

# strip tile splits, in super-rows (see module docstring)
S1CUTS = [(0, 18), (15, 32)]
S2CUTS = [(0, 18), (15, 25), (23, 32)]
W2T_1 = [0] * 11 + [1] * 11              # window -> s1 tile
W2T_2 = [0] * 11 + [1] * 5 + [2] * 6     # window -> s2 tile


def _slen(cut):
    n = GF + (cut[1] - cut[0]) * SR + 288
    return ((n + 15) // 16) * 16


S1LENS = [_slen(c) for c in S1CUTS]
S2LENS = [_slen(c) for c in S2CUTS]


def route(cuts, ra, rb):
    """Map super-row range [ra, rb) onto the strip tiles."""
    out = []
    for t, (lo, hi) in enumerate(cuts):
        a, b = max(ra, lo), min(rb, hi)
        if a < b:
            out.append((t, a, b))
    return out


# windows: (e0, ln); 21 full 396-col windows + one 132-col tail
WINDOWS = []
_e = 0
while _e < STRIP:
    ln = min(WCOLS, STRIP - _e)
    WINDOWS.append((_e, ln))
    _e += ln

# pairs: 2 windows each -> super-row aligned (3 rows; last pair 2 rows)
PAIRS = []
for _i in range(0, len(WINDOWS), 2):
    ws_ = WINDOWS[_i:_i + 2]
    e0 = ws_[0][0]
    ln = sum(w[1] for w in ws_)
    assert e0 % SR == 0 and ln % SR == 0
    PAIRS.append((list(range(_i, _i + len(ws_))), e0, ln,
                  e0 // SR, (e0 + ln) // SR))

# per-channel param column order inside the packed [128, 22] table
PARAM_ORDER = [
    "b1", "alpha", "bn1_gamma", "bn1_beta", "bn1_mean", "bn1_var",
    "b2", "bn2_gamma", "bn2_beta", "bn2_mean", "bn2_var",
]
NPARAM = len(PARAM_ORDER)


def _rows_ap(t2d, a, b, base=0):
    """[128, b-a, 8, 32] AP over real cols of super-rows [a, b)."""
    ap = t2d[:, base + a * SR: base + b * SR]
    ap = ap.rearrange("p (r i c) -> p r i c", i=NIMG, c=NCOL)
    return ap[:, :, :, :32]


def build_program():
    nc = bacc.Bacc("TRN2", target_bir_lowering=False, debug=False,
                   num_devices=NCORES)

    xs = nc.declare_dram_parameter("xs", [2, 128, STRIP], F32, isOutput=False)
    w1 = nc.declare_dram_parameter("w1", [128, 18, 2, 128], BF16,
                                   isOutput=False)
    w2 = nc.declare_dram_parameter("w2", [128, 18, 2, 128], BF16,
                                   isOutput=False)
    pv = nc.declare_dram_parameter("pv", [128, 2 * NPARAM], F32, isOutput=False)
    outd = nc.declare_dram_parameter("out", [2, 128, STRIP], BF16,
                                     isOutput=True)

    with tile.TileContext(nc) as tc:
        with (
            tc.tile_pool(name="big", bufs=1) as big,
            tc.tile_pool(name="wstage", bufs=2) as wsp,
            tc.tile_pool(name="xw", bufs=6) as xwp,
            tc.tile_pool(name="t2", bufs=4) as t2p,
            tc.tile_pool(name="psum", bufs=8, space="PSUM") as psp,
        ):
            s1t = [big.tile([128, 2, L], F8, tag=f"s1t{i}", name=f"s1t{i}")
                   for i, L in enumerate(S1LENS)]
            s2t = [big.tile([128, 2, L], F8, tag=f"s2t{i}", name=f"s2t{i}")
                   for i, L in enumerate(S2LENS)]
            o1p = [big.tile([128, 2, p[2]], F32, tag=f"o1p{k}",
                            name=f"o1p{k}")
                   for k, p in enumerate(PAIRS)]
            w1f = [big.tile([128, 9, 2, 128], F8, tag=f"w1f{m}",
                            name=f"w1f{m}") for m in range(2)]
            w2f = [big.tile([128, 9, 2, 128], F8, tag=f"w2f{m}",
                            name=f"w2f{m}") for m in range(2)]
            pt = big.tile([128, 2 * NPARAM], F32, tag="pt")
            dv = big.tile([128, 14], F32, tag="dv")
            scr = big.tile([128, 8], F32, tag="scr")
            scr2 = big.tile([128, 8], F32, tag="scr2")

            def w1ap(m, tap):
                return w1f[m][:, tap, :, :]

            def w2ap(m, tap):
                return w2f[m][:, tap, :, :]

            # dummy ACTIVATE with no data deps: forces the ACT table load
            # to happen immediately instead of before the first real sign
            nc.scalar.sign(out=scr2[:, 6:7],
               in_=nc.const_aps.tensor(0.0, (128, 1)))

            # s1 guard zeroing FIRST on DVE so it never gates the first
            # matmul behind the binarize chain (interior is fully
            # overwritten by sign1, pad cols included: host x strip has
            # zero pads, sign(0)=0)
            U32 = mybir.dt.uint32
            for i, st in enumerate(s1t):
                dlen = (S1CUTS[i][1] - S1CUTS[i][0]) * SR
                nc.vector.memset(st[:, :, 0:GF].bitcast(U32), 0)
                nc.vector.memset(st[:, :, GF + dlen:S1LENS[i]], 0.0)

            # ---- params first: everything derived hangs off this DMA --
            nc.gpsimd.dma_start(out=pt[:, :], in_=pv[:, :])

            # conv1 weights (gate the first matmuls): DMA via the SP queue
            # (parallel with GpSimd's x feed), VectorE binarize to
            # (w>0)-0.5 = +-0.5, per m-half so conv m=0 only waits its own
            for m in range(2):
                ws = wsp.tile([128, 9, 2, 128], BF16, tag="ws", name="ws")
                nc.sync.dma_start(out=ws[:, :, :, :],
                                  in_=w1[:, m * 9:(m + 1) * 9, :, :])
                nc.vector.tensor_scalar(w1f[m][:, :, :, :],
                                        ws[:, :, :, :], 0.0, 0.5,
                                        AOP.is_gt, AOP.subtract)

            # ---- x pair feed: DMA (GpSimd queue) + ScalarE sign -> s1 --
            xwt = {}

            def feed_pair(pi):
                if pi in xwt or pi >= len(PAIRS):
                    return
                _wis, e0, ln, r0, r1 = PAIRS[pi]
                xb = xwp.tile([128, 2, 3 * SR], F32, tag="xw", name="xw")
                for i in range(2):
                    nc.gpsimd.dma_start(out=xb[:, i, :ln],
                                        in_=xs[i, :, e0:e0 + ln])
                for t, lo, hi in route(S1CUTS, r0, r1):
                    o0 = GF + (lo - S1CUTS[t][0]) * SR
                    nc.scalar.sign(
                        out=s1t[t][:, :, o0: o0 + (hi - lo) * SR],
                        in_=xb[:, :, (lo - r0) * SR:(hi - r0) * SR])
                xwt[pi] = xb

            for pi in range(2):
                feed_pair(pi)

            def pcol(m, name):
                k = PARAM_ORDER.index(name)
                return pt[:, m * NPARAM + k: m * NPARAM + k + 1]

            def dcol(j):
                return dv[:, j: j + 1]

            # Batched rsqrt(var+eps) for all 4 (conv, half) columns at
            # once: Quake-III bit-trick seed + 3 Newton iterations, all on
            # VectorE (no ScalarE Sqrt -> no extra ACT table load).
            vco = [("bn1", 0), ("bn1", 1), ("bn2", 0), ("bn2", 1)]
            vpe = scr[:, 0:4]
            for j, (pfx, m) in enumerate(vco):
                nc.vector.tensor_scalar_add(scr[:, j:j + 1],
                                            pcol(m, pfx + "_var"), EPS)
            yb = scr[:, 4:8]
            nc.vector.memset(yb.bitcast(U32), 0x5f3759df)
            nc.vector.tensor_scalar(scr2[:, 0:4].bitcast(U32),
                                    vpe.bitcast(U32), 1, None,
                                    AOP.logical_shift_right)
            nc.vector.tensor_tensor(yb.bitcast(U32), yb.bitcast(U32),
                                    scr2[:, 0:4].bitcast(U32), AOP.subtract)
            for _ in range(3):
                nc.vector.tensor_tensor(scr2[:, 0:4], yb, yb, AOP.mult)
                nc.vector.tensor_tensor(scr2[:, 0:4], vpe, scr2[:, 0:4],
                                        AOP.mult)
                nc.vector.tensor_scalar(scr2[:, 0:4], scr2[:, 0:4], -0.5, 1.5,
                                        AOP.mult, AOP.add)
                nc.vector.tensor_tensor(yb, yb, scr2[:, 0:4], AOP.mult)

            # dv columns: per conv c (0/1), half m: sc = c*6+m, g*b = c*6+2+m,
            # d = c*6+4+m; 12+m = -d2 (sign2 bias)
            for j, (pfx, m) in enumerate(vco):
                ci = j // 2
                gam = pcol(m, pfx + "_gamma")
                bet = pcol(m, pfx + "_beta")
                mean = pcol(m, pfx + "_mean")
                bvec = pcol(m, "b1" if ci == 0 else "b2")
                rs = yb[:, j:j + 1]
                g = scr2[:, 4:5]
                nc.vector.tensor_tensor(g, gam, rs, AOP.mult)
                nc.vector.tensor_scalar_mul(dcol(ci * 6 + m), g, STD * 2.0)
                nc.vector.tensor_tensor(dcol(ci * 6 + 2 + m), g, bvec, AOP.mult)
                nc.vector.tensor_tensor(scr2[:, 5:6], mean, g, AOP.mult)
                nc.vector.tensor_tensor(dcol(ci * 6 + 4 + m), bet,
                                        scr2[:, 5:6], AOP.subtract)
            # o1 stores out1 + d2 (both BN shifts folded into conv1's single
            # post-add): conv1 d column := d1 + d2, and sign(out1) =
            # Sign(o1 - d2) needs -d2 as an ACT bias column.
            for m in range(2):
                nc.vector.tensor_scalar_mul(dcol(12 + m), dcol(10 + m), -1.0)
                nc.vector.tensor_tensor(dcol(4 + m), dcol(4 + m),
                                        dcol(10 + m), AOP.add)

            def sc_ap(conv, m):
                return dcol((conv - 1) * 6 + m)

            def bi_ap(conv, m):
                return dcol((conv - 1) * 6 + 2 + m)

            def dd_ap(conv, m):
                return dcol((conv - 1) * 6 + 4 + m)

            def nd2_ap(m):
                return dcol(12 + m)

            def al_ap(m):
                return pcol(m, "alpha")

            # s2 cleared after the startup-critical work is queued
            for st in s2t:
                for i in range(2):
                    nc.vector.memset(st[:, i, :].bitcast(U32), 0)
            # conv2 weights
            for m in range(2):
                ws = wsp.tile([128, 9, 2, 128], BF16, tag="ws", name="ws")
                nc.sync.dma_start(out=ws[:, :, :, :],
                                  in_=w2[:, m * 9:(m + 1) * 9, :, :])
                nc.vector.tensor_scalar(w2f[m][:, :, :, :],
                                        ws[:, :, :, :], 0.0, 0.5,
                                        AOP.is_gt, AOP.subtract)

            # ---- the two convs: matmuls issued in 2-pair tap blocks so
            # each LDWEIGHTS covers 4 matmuls (hides the weight load);
            # post-ops stay per-pair (independent dependency chains) ------
            def conv_super(stiles, cuts, w2t, wap, convno, prs):
                first = convno == 1
                for m in range(2):
                    ps = {}
                    for pr in prs:
                        for wi in pr[0]:
                            ps[wi] = psp.tile([128, WCOLS], F32,
                                              tag="ps", name="ps")
                    for tap in range(9):
                        dy, dx = divmod(tap, 3)
                        off = (dy - 1) * SR + (dx - 1)
                        lhsT = wap(m, tap)
                        for pr in prs:
                            for wi in pr[0]:
                                we0, wln = WINDOWS[wi]
                                t = w2t[wi]
                                st = stiles[t]
                                c0 = GF + (we0 - cuts[t][0] * SR) + off
                                nc.tensor.matmul(
                                    ps[wi][:, :wln], lhsT,
                                    st[:, :, c0: c0 + wln],
                                    start=(tap == 0), stop=(tap == 8),
                                    perf_mode=DR)
                    for pr in prs:
                        wis, e0, ln, r0, r1 = pr
                        pi = e0 // (3 * SR)
                        if first:
                            dst = o1p[pi][:, m, :ln]
                        else:
                            tb = t2p.tile([128, 3 * SR], BF16, tag="t2",
                                          name="t2")
                            dst = tb[:, :ln]
                        o_off = 0
                        for wi in wis:
                            wln = WINDOWS[wi][1]
                            if first or m == 0:
                                nc.scalar.activation(
                                    dst[:, o_off:o_off + wln],
                                    ps[wi][:, :wln], AFT.Identity,
                                    bias=bi_ap(convno, m),
                                    scale=sc_ap(convno, m))
                            else:
                                # conv2 m=1 evac on DVE: halves the serial
                                # ScalarE chain per pair and the final drain
                                nc.vector.tensor_scalar(
                                    dst[:, o_off:o_off + wln],
                                    ps[wi][:, :wln],
                                    sc_ap(convno, m), bi_ap(convno, m),
                                    AOP.mult, AOP.add)
                            o_off += wln
                        # PReLU + residual add on DVE (conv1 f32, conv2 bf16)
                        nc.vector.scalar_tensor_tensor(
                            dst, dst, al_ap(m), dst, AOP.mult, AOP.max)
                        if first:
                            nc.vector.scalar_tensor_tensor(
                                dst, dst, dd_ap(convno, m),
                                xwt[pi][:, m, :ln], AOP.add, AOP.add)
                        else:
                            nc.vector.tensor_tensor(
                                dst, dst, o1p[pi][:, m, :ln], AOP.add)
                            nc.sync.dma_start(
                                out=outd[m, :, e0:e0 + ln], in_=dst)
                if first:
                    # sign(out1) = Sign(o1 - d2) on ScalarE, interior-only
                    for pr in prs:
                        wis, e0, ln, r0, r1 = pr
                        pi = e0 // (3 * SR)
                        for m in range(2):
                            for t, lo, hi in route(S2CUTS, r0, r1):
                                base = S2CUTS[t][0]
                                nc.scalar.activation(
                                    _rows_ap(s2t[t][:, m], lo - base,
                                             hi - base, base=GF),
                                    _rows_ap(o1p[pi][:, m], lo - r0, hi - r0),
                                    AFT.Sign, bias=nd2_ap(m), scale=1.0)

            feed_pair(2)
            feed_pair(3)
            sgroups = [PAIRS[g:g + 2] for g in range(0, len(PAIRS), 2)]
            for gi, prs in enumerate(sgroups):
                conv_super(s1t, S1CUTS, W2T_1, w1ap, 1, prs)
                feed_pair(2 * gi + 4)
                feed_pair(2 * gi + 5)
            for prs in sgroups:
                conv_super(s2t, S2CUTS, W2T_2, w2ap, 2, prs)

    nc.compile()
    return nc


# ---------------------------------------------------------------- host side

def _host_pack_x(x_shard):
    """[8,256,32,32] f32 -> strip layout [2,128,STRIP] with zero pad cols."""
    xz = np.zeros((2, 128, NSR, NIMG, NCOL), dtype=np.float32)
    xr = x_shard.reshape(NIMG, 2, 128, 32, 32)
    xz[:, :, :, :, :32] = xr.transpose(1, 2, 3, 0, 4)
    return np.ascontiguousarray(xz.reshape(2, 128, STRIP))


def _host_pack_w(W):
    """[256,256,3,3] -> [128(k), 18(m*9+tap), 2(i), 128(j)] bf16.

    bf16 preserves the sign of every f32 exactly (same exponent range, no
    flush to zero), and the kernel only uses sign(w)."""
    import ml_dtypes
    A = np.asarray(W, dtype=np.float32).reshape(2, 128, 2, 128, 3, 3)
    L = A.transpose(3, 4, 5, 0, 2, 1)          # (k, dy, dx, m, i, j)
    L = L.reshape(128, 9, 2, 2, 128)           # (k, tap, m, i, j)
    L = L.transpose(0, 2, 1, 3, 4)             # (k, m, tap, i, j)
    return np.ascontiguousarray(L.reshape(128, 18, 2, 128)
                                .astype(ml_dtypes.bfloat16))


def _host_pack_pv(inputs):
    pvt = np.zeros((128, 2 * NPARAM), dtype=np.float32)
    for k, name in enumerate(PARAM_ORDER):
        v = np.asarray(inputs[name], dtype=np.float32)
        for m in range(2):
            pvt[:, m * NPARAM + k] = v[m * 128:(m + 1) * 128]
    return pvt


def _host_unpack_out(o):
    """[2,128,STRIP] bf16 -> [8,256,32,32] f32."""
    o = np.asarray(o, dtype=np.float32)
    o = o.reshape(2, 128, NSR, NIMG, NCOL)[:, :, :, :, :32]
    return np.ascontiguousarray(o.transpose(3, 0, 1, 2, 4)
                                .reshape(NIMG, 256, 32, 32))


_PROG = None
LAST_EXEC_TIME_NS = None
LAST_RESULT = None


def _get_prog():
    global _PROG
    if _PROG is None:
        _PROG = build_program()
    return _PROG


def kernel(x, W1, b1, W2, b2, alpha,
           bn1_gamma, bn1_beta, bn1_mean, bn1_var,
           bn2_gamma, bn2_beta, bn2_mean, bn2_var,
           _trace=False):
    global LAST_EXEC_TIME_NS
    global LAST_RESULT
    inputs = dict(b1=b1, b2=b2, alpha=alpha,
                  bn1_gamma=bn1_gamma, bn1_beta=bn1_beta,
                  bn1_mean=bn1_mean, bn1_var=bn1_var,
                  bn2_gamma=bn2_gamma, bn2_beta=bn2_beta,
                  bn2_mean=bn2_mean, bn2_var=bn2_var)
    x = np.asarray(x, dtype=np.float32)
    w1l = _host_pack_w(W1)
    w2l = _host_pack_w(W2)
    pvt = _host_pack_pv(inputs)

    in_maps = []
    for c in range(NCORES):
        shard = x[c * NIMG:(c + 1) * NIMG]
        in_maps.append({"xs": _host_pack_x(shard), "w1": w1l, "w2": w2l,
                        "pv": pvt})

    nc = _get_prog()
    res = run_bass_kernel_spmd(nc, in_maps, core_ids=list(range(NCORES)),
                               trace=_trace)
    LAST_EXEC_TIME_NS = res.exec_time_ns
    LAST_RESULT = res

    outs = [_host_unpack_out(res.results[c]["out"]) for c in range(NCORES)]
    return np.concatenate(outs, axis=0)
